# revision 1
# baseline (speedup 1.0000x reference)
"""Causal self-attention (B=2, S=2048, D=1024, H=16, Dh=64) on 8 trn2 cores.

Sharding: data-parallel over batch (2 groups of 4 cores) x tensor-parallel over
heads (4 heads/core). Each core computes its heads' attention and a partial
c_proj product; the host sums the 4 partials per batch and adds b_proj.

Per-core device kernel (all layouts chosen so no on-device transposes of x are
needed; the only transposes are 32 small PE transposes of the attention output):
  - QKV proj: Q^T,K^T computed in [dh, S] layout (lhsT=Wqk, rhs=x^T), V in
    natural [S, dh] layout (lhsT=x^T, rhs=Wv) + rank-1 bias via ones-column
    matmul. fp32r matmuls (TF32-like, 1 cyc/row at N>=256).
  - scores^T in [k, q] layout; exp on ScalarE with scale=1/8, output bf16;
    causal mask applied multiplicatively on diagonal tiles only.
  - PV: out[q128, 65] = sum_k phat^T.T @ [V | 1]; column 64 of the ones-padded
    V picks up the softmax denominator for free. Normalize with per-partition
    reciprocal multiply.
  - proj: PE-transpose O to O^T, then partial y = O^T.T @ Wp_slice (fp32r).
"""

import os
import sys

for _p in ("/opt/trn_rl_repo", "/root/.axon_site/_ro/trn_rl_repo"):
    if os.path.isdir(_p) and _p not in sys.path:
        sys.path.insert(0, _p)

import numpy as np
import ml_dtypes

import concourse.bacc as bacc
import concourse.tile as tile
from concourse import mybir
from concourse.bass_utils import run_bass_kernel_spmd
from concourse.masks import make_identity

F32 = mybir.dt.float32
F32R = mybir.dt.float32r
BF16 = mybir.dt.bfloat16

B, S, D, H, DH = 2, 2048, 1024, 16, 64
HC = 4          # heads per core
EQK = 512       # q+k weight cols per core (2*HC*DH)
EV = 256        # v weight cols per core (HC*DH)
ND = D // 128   # 8 d-tiles
NS = S // 128   # 16 s-tiles (also k-tiles)
NQ = S // 512   # 4 q512-tiles


def build_nc():
    nc = bacc.Bacc("TRN2", target_bir_lowering=False, debug=False)

    xT = nc.dram_tensor("xT", [D, S], F32, kind="ExternalInput").ap()
    wqk = nc.dram_tensor("wqk", [D, EQK], F32, kind="ExternalInput").ap()
    bqk = nc.dram_tensor("bqk", [128, 4], F32, kind="ExternalInput").ap()
    wv = nc.dram_tensor("wv", [D, EV], F32, kind="ExternalInput").ap()
    bv = nc.dram_tensor("bv", [1, EV], F32, kind="ExternalInput").ap()
    wp = nc.dram_tensor("wp", [EV, D], F32, kind="ExternalInput").ap()
    masks = nc.dram_tensor("masks", [128, 4, 512], BF16, kind="ExternalInput").ap()
    y = nc.dram_tensor("y", [S, D], F32, kind="ExternalOutput").ap()

    with tile.TileContext(nc) as tc:
        _emit(nc, tc, xT, wqk, bqk, wv, bv, wp, masks, y)
    nc.compile()
    return nc


def _emit(nc, tc, xT, wqk, bqk, wv, bv, wp, masks, y):
    from contextlib import ExitStack

    with ExitStack() as top:
        consts = top.enter_context(tc.tile_pool(name="consts", bufs=1))
        acts = top.enter_context(tc.tile_pool(name="acts", bufs=1))

        identity = consts.tile([128, 128], F32)
        make_identity(nc, identity)
        ones_f32 = consts.tile([1, 128], F32)
        nc.vector.memset(ones_f32, 1.0)
        ones_col = consts.tile([1, 128], F32R)
        nc.vector.tensor_copy(ones_col[:], ones_f32[:])
        masks_sb = consts.tile([128, 4, 512], BF16)
        nc.sync.dma_start(masks_sb[:], masks[:])
        bqk_sb = consts.tile([128, 4], F32)
        nc.sync.dma_start(bqk_sb[:], bqk[:])
        bv_sb = consts.tile([1, EV], F32R)
        nc.sync.dma_start(bv_sb[:], bv[:].bitcast(F32R))
        wp_sb = consts.tile([128, 2, D], F32R)
        for t in range(2):
            nc.sync.dma_start(wp_sb[:, t, :], wp[128 * t:128 * (t + 1), :].bitcast(F32R))

        # persistent activations
        # Q^T/K^T, [dh, S] layout, head-pairs packed on partitions (0:64 / 64:128)
        qt_sb = [acts.tile([128, S], F32R, tag=f"qt{hp}", name=f"qt{hp}") for hp in range(2)]
        kt_sb = [acts.tile([128, S], F32R, tag=f"kt{hp}", name=f"kt{hp}") for hp in range(2)]
        # [V | 1] per head per k-tile: [128, h, kt, 65] bf16
        vhat = acts.tile([128, HC, NS, DH + 1], BF16, tag="vhat")
        # attention output, natural [q, feat] layout, head pairs side by side
        o_sb = acts.tile([128, 2, NS, 128], F32, tag="o_sb")
        ot_sb = acts.tile([128, 2, S], F32R, tag="ot_sb")  # O^T [feat, s] for proj

        # ---------------- Phase A: QKV projections ----------------
        with ExitStack() as pa:
            xt_pool = pa.enter_context(tc.tile_pool(name="xt", bufs=1))
            w_pool = pa.enter_context(tc.tile_pool(name="w", bufs=1))
            ps_qk = pa.enter_context(tc.tile_pool(name="ps_qk", bufs=5, space="PSUM"))
            ps_v = pa.enter_context(tc.tile_pool(name="ps_v", bufs=3, space="PSUM"))

            xt_sb = xt_pool.tile([128, ND, S], F32R)
            wqk_sb = w_pool.tile([128, ND, EQK], F32R)
            wv_sb = w_pool.tile([128, ND, EV], F32R)
            # interleave so the QK dt-chain can start after the first pair lands
            for t in range(ND):
                nc.sync.dma_start(wqk_sb[:, t, :], wqk[128 * t:128 * (t + 1), :].bitcast(F32R))
                nc.sync.dma_start(xt_sb[:, t, :], xT[128 * t:128 * (t + 1), :].bitcast(F32R))
            for t in range(ND):
                nc.sync.dma_start(wv_sb[:, t, :], wv[128 * t:128 * (t + 1), :].bitcast(F32R))

            # Q^T/K^T: psum[e128, s512] = sum_d wqk[d,e].T @ xT[d,s]
            # e-tile order: 0 -> Q hp0, 1 -> K hp0, 2 -> Q hp1, 3 -> K hp1
            # dt is the OUTER loop so compute starts when the first d-tile lands
            for et in range(4):
                dest = (qt_sb if et % 2 == 0 else kt_sb)[et // 2]
                ps = [ps_qk.tile([128, 512], F32, tag="pqk", name=f"pqk{st}") for st in range(NQ)]
                for dt in range(ND):
                    for st in range(NQ):
                        nc.tensor.matmul(
                            ps[st][:],
                            wqk_sb[:, dt, 128 * et:128 * (et + 1)],
                            xt_sb[:, dt, 512 * st:512 * (st + 1)],
                            start=(dt == 0), stop=(dt == ND - 1),
                        )
                for st in range(NQ):
                    nc.vector.tensor_scalar_add(
                        dest[:, 512 * st:512 * (st + 1)], ps[st][:], bqk_sb[:, et:et + 1]
                    )

            # V natural: psum[s128, 256] = sum_d xT[d,s].T @ wv[d,:] (+ ones x bv)
            for st in range(NS):
                p = ps_v.tile([128, EV], F32)
                for dt in range(ND):
                    nc.tensor.matmul(
                        p[:],
                        xt_sb[:, dt, 128 * st:128 * (st + 1)],
                        wv_sb[:, dt, :],
                        start=(dt == 0), stop=False,
                    )
                nc.tensor.matmul(p[:], ones_col[:], bv_sb[:], start=False, stop=True)
                nc.vector.tensor_copy(
                    vhat[:, :, st, 0:DH],
                    p[:].rearrange("p (h e) -> p h e", h=HC),
                )
            nc.vector.memset(vhat[:, :, :, DH:DH + 1], 1.0)

        # ---------------- Phase B: attention ----------------
        # scores^T/exp for iteration i are interleaved with the PV/transpose
        # work of iteration i-1 so PE keeps streaming while ScalarE runs exp.
        with ExitStack() as pb:
            ps_sc = pb.enter_context(tc.tile_pool(name="ps_sc", bufs=2, space="PSUM"))
            ps_ot = pb.enter_context(tc.tile_pool(name="ps_ot", bufs=1, space="PSUM"))
            ps_tp = pb.enter_context(tc.tile_pool(name="ps_tp", bufs=1, space="PSUM"))
            ps_y = pb.enter_context(tc.tile_pool(name="ps_y", bufs=2, space="PSUM"))
            outp = pb.enter_context(tc.tile_pool(name="outp", bufs=4))
            phat_pool = pb.enter_context(tc.tile_pool(name="phat", bufs=2))
            otsb_pool = pb.enter_context(tc.tile_pool(name="otsb", bufs=3))
            small = pb.enter_context(tc.tile_pool(name="small", bufs=8))

            def emit_pv(hp, qt, phat):
                """PV (streaming, lhsT=V stationary) + transpose-back + normalize.
                out^T[65, q512] = sum_kt vhat[:,h,kt,:].T @ phat[kt, q]; row 64
                is the softmax denominator. Transpose 128-blocks back to
                [q, 65], then reciprocal-scale into o_sb. Yields between
                instructions so the caller can interleave emission."""
                nkt = 4 * (qt + 1)
                for h in range(2):
                    hg = 2 * hp + h
                    po = ps_ot.tile([65, 512], F32, tag="ot", name="po")
                    for kt in range(nkt):
                        nc.tensor.matmul(
                            po[:],
                            vhat[:, hg, kt, :],
                            phat[h][:, 512 * kt:512 * (kt + 1)],
                            start=(kt == 0), stop=(kt == nkt - 1),
                        )
                        yield
                    so = otsb_pool.tile([65, 512], F32, tag="so", name="so")
                    nc.vector.tensor_copy(so[:], po[:])
                    yield
                    for b4 in range(4):
                        jq = 4 * qt + b4
                        pt = ps_tp.tile([128, DH + 1], F32, tag="tp", name="pt")
                        nc.tensor.transpose(
                            pt[:], so[:, 128 * b4:128 * (b4 + 1)], identity[0:65, 0:65]
                        )
                        recip = small.tile([128, 1], F32)
                        nc.vector.reciprocal(recip[:], pt[:, DH:DH + 1])
                        nc.vector.tensor_scalar_mul(
                            o_sb[:, hp, jq, 64 * h:64 * (h + 1)], pt[:, 0:DH], recip[:]
                        )
                        yield
                # both heads normalized: feed the proj-prep transposes now so
                # the kernel tail is just the proj matmuls
                for b4 in range(4):
                    jq = 4 * qt + b4
                    pt2 = ps_tp.tile([128, 128], F32, tag="tp", name="pt2")
                    nc.tensor.transpose(pt2[:], o_sb[:, hp, jq, :], identity[:])
                    nc.vector.tensor_copy(ot_sb[:, hp, 128 * jq:128 * (jq + 1)], pt2[:])
                    yield
                if hp == 1:
                    # both feature halves of ot_sb are now final for these s
                    # tiles: emit their slice of the projection
                    for b4 in range(4):
                        st = 4 * qt + b4
                        for nt in range(2):
                            py = ps_y.tile([128, 512], F32, tag="py", name="py")
                            for ft in range(2):
                                nc.tensor.matmul(
                                    py[:],
                                    ot_sb[:, ft, 128 * st:128 * (st + 1)],
                                    wp_sb[:, ft, 512 * nt:512 * (nt + 1)],
                                    start=(ft == 0), stop=(ft == 1),
                                )
                            ys = outp.tile([128, 512], F32, tag="ys", name="ys")
                            nc.vector.tensor_copy(ys[:], py[:])
                            nc.sync.dma_start(
                                y[128 * st:128 * (st + 1), 512 * nt:512 * (nt + 1)], ys[:]
                            )
                            yield

            prev = iter(())
            for hp in range(2):
                for qt in range(NQ):
                    nkt = 4 * (qt + 1)  # k-tiles in causal range
                    ngrp = nkt // 2
                    phat = [phat_pool.tile([128, NS * 512], BF16, tag=f"phat{h}", name=f"phat{h}") for h in range(2)]
                    for g in range(ngrp):
                        psc = [ps_sc.tile([128, 1024], F32, tag="sc", name=f"sc{h}") for h in range(2)]
                        for j in range(2):
                            kt = 2 * g + j
                            for h in range(2):
                                sl = slice(64 * h, 64 * (h + 1))
                                nc.tensor.matmul(
                                    psc[h][:, 512 * j:512 * (j + 1)],
                                    kt_sb[hp][sl, 128 * kt:128 * (kt + 1)],
                                    qt_sb[hp][sl, 512 * qt:512 * (qt + 1)],
                                    start=True, stop=True,
                                )
                        for h in range(2):
                            nc.scalar.activation(
                                phat[h][:, 1024 * g:1024 * (g + 1)],
                                psc[h][:],
                                mybir.ActivationFunctionType.Exp,
                                scale=0.125,
                            )
                            # causal mask on diagonal k-tiles, right after exp
                            for j in range(2):
                                kt = 2 * g + j
                                if kt >= 4 * qt:
                                    sl = slice(512 * kt, 512 * (kt + 1))
                                    nc.vector.tensor_mul(
                                        phat[h][:, sl], phat[h][:, sl],
                                        masks_sb[:, kt - 4 * qt, :],
                                    )
                        # interleave a slice of the previous iteration's PV work
                        nchunk = 20 if (hp == 1 and qt == NQ - 1) else 12
                        for _ in range(nchunk):
                            if next(prev, None) is None:
                                break
                    for _ in prev:
                        pass  # drain any leftover PV work before swapping
                    prev = emit_pv(hp, qt, phat)
            for _ in prev:
                pass


_NC = None


def _get_nc():
    global _NC
    if _NC is None:
        _NC = build_nc()
    return _NC


def _make_masks():
    i = np.arange(128)[:, None]
    j = np.arange(512)[None, :]
    m = np.stack([(i + 128 * o <= j) for o in range(4)], axis=1)  # [128, 4, 512]
    return m.astype(ml_dtypes.bfloat16)


def _in_maps(x, W_attn, b_attn, W_proj):
    masks = _make_masks()
    maps = []
    for c in range(8):
        b, g = c // 4, c % 4
        heads = [4 * g + i for i in range(HC)]
        qc = [W_attn[:, 64 * h:64 * (h + 1)] for h in heads]
        kc = [W_attn[:, D + 64 * h:D + 64 * (h + 1)] for h in heads]
        vc = [W_attn[:, 2 * D + 64 * h:2 * D + 64 * (h + 1)] for h in heads]
        bq = [b_attn[64 * h:64 * (h + 1)] for h in heads]
        bk = [b_attn[D + 64 * h:D + 64 * (h + 1)] for h in heads]
        bvs = [b_attn[2 * D + 64 * h:2 * D + 64 * (h + 1)] for h in heads]
        wqk_c = np.ascontiguousarray(np.concatenate(
            [qc[0], qc[1], kc[0], kc[1], qc[2], qc[3], kc[2], kc[3]], axis=1))
        bqk_c = np.concatenate(
            [bq[0], bq[1], bk[0], bk[1], bq[2], bq[3], bk[2], bk[3]])
        bqk_c = np.ascontiguousarray(bqk_c.reshape(4, 128).T)
        wv_c = np.ascontiguousarray(np.concatenate(vc, axis=1))
        bv_c = np.ascontiguousarray(np.concatenate(bvs).reshape(1, EV))
        wp_c = np.ascontiguousarray(W_proj[EV * g:EV * (g + 1), :])
        xT_c = np.ascontiguousarray(x[b].T)
        maps.append({
            "xT": xT_c, "wqk": wqk_c, "bqk": bqk_c, "wv": wv_c,
            "bv": bv_c, "wp": wp_c, "masks": masks,
        })
    return maps


def _gather(results, b_proj):
    y = np.empty((B, S, D), np.float32)
    for b in range(B):
        acc = results[4 * b]["y"].astype(np.float32).copy()
        for g in range(1, 4):
            acc += results[4 * b + g]["y"]
        y[b] = acc + b_proj[None, :]
    return y


def run(x, W_attn, b_attn, W_proj, b_proj, trace=False):
    x = np.asarray(x, np.float32)
    W_attn = np.asarray(W_attn, np.float32)
    b_attn = np.asarray(b_attn, np.float32)
    W_proj = np.asarray(W_proj, np.float32)
    b_proj = np.asarray(b_proj, np.float32)
    nc = _get_nc()
    res = run_bass_kernel_spmd(nc, _in_maps(x, W_attn, b_attn, W_proj),
                               core_ids=list(range(8)), trace=trace)
    return _gather(res.results, b_proj), res


def kernel(x, W_attn, b_attn, W_proj, b_proj):
    out, _ = run(x, W_attn, b_attn, W_proj, b_proj)
    return out

